# revision 16
# baseline (speedup 1.0000x reference)
"""Batched ragged segment-mean (BERTEmbedder merge loop) on 8 TRN2 NeuronCores.

Strategy
--------
Data-parallel over the batch: core c gets sequences [2c, 2c+1].  Within a
sequence, segment-sum is computed as a block-sparse one-hot matmul on the PE:

    out[t, d] = sum_s onehot[s, t] * x[s, d]

Segment ids are sorted per row, so each 128-subtoken tile only covers a
narrow window of token ids.  The host inspects the ids and builds a static
(s_tile, t_tile) pair schedule: for each 128-wide s-tile we emit matmuls only
into the 128-row t-tiles its ids can touch (union over the sequences that
share the SPMD program slot, so one program serves all 8 cores).  A column of
ones appended to the moving operand accumulates per-token counts in the same
PSUM tile; a reciprocal-multiply then turns sums into means.

fp32 matmul on TRN2 costs 4 PE cycles per output row (two half-speed passes).
Instead we run two 1-cycle fp32r matmuls: hi = round_fp32r(x) (11 mantissa
bits survive, measured) and lo = round_fp32r(x - hi), accumulated into the
same fp32 PSUM — reconstructing ~22 mantissa bits, indistinguishable from
fp32 at the output tolerance.  The one-hot (exactly representable) is built
on GPSIMD from a static iota and the per-partition segment id; segment ids
reach the partition dimension via one PE transpose per sequence.
"""

import os
import numpy as np

B, S, D, T, P = 16, 4096, 768, 2048, 128
NCORES = 8
SPC = B // NCORES          # sequences per core
NST, NTT = S // P, T // P  # 32 s-tiles, 16 t-tiles
DSPLIT = 512               # PSUM bank limit (fp32 words)
DW = D + 2                 # data + count-ones col + pad col (fp32r needs even N)
SUPER = 2                  # s-tiles per x-load DMA

_cache: dict = {}


def _schedule(segment_ids: np.ndarray):
    """Per program slot q: which t-tiles each s-tile touches, unioned over the
    sequences that run in that slot on every core (SPMD: one program)."""
    sched = []
    for q in range(SPC):
        seqs = [c * SPC + q for c in range(NCORES)]
        js_of = []
        for i in range(NST):
            blk = segment_ids[seqs, i * P:(i + 1) * P]
            lo, hi = int(blk.min()), int(blk.max())
            js_of.append(list(range(lo // P, hi // P + 1)))
        first, last = {}, {}
        for i in range(NST):
            for j in js_of[i]:
                first.setdefault(j, i)
                last[j] = i
        sched.append((tuple(tuple(js) for js in js_of),
                      tuple(sorted(first.items())),
                      tuple(sorted(last.items()))))
    return tuple(sched)


def _build(sched):
    from contextlib import ExitStack
    import concourse.bacc as bacc
    import concourse.tile as tile
    import concourse.mybir as mybir

    f32, f32r, i32 = mybir.dt.float32, mybir.dt.float32r, mybir.dt.int32
    AO = mybir.AluOpType
    nc = bacc.Bacc("TRN2", target_bir_lowering=False, debug=False)
    x = nc.dram_tensor("raw_output", [SPC, S, D], f32, kind="ExternalInput").ap()
    sid = nc.dram_tensor("segment_ids", [SPC, S], i32, kind="ExternalInput").ap()
    out = nc.dram_tensor("out", [SPC, T, D], f32, kind="ExternalOutput").ap()

    with ExitStack() as ctx:
        tc = ctx.enter_context(tile.TileContext(nc))
        const = ctx.enter_context(tc.tile_pool(name="const", bufs=1))
        xp = ctx.enter_context(tc.tile_pool(name="xp", bufs=6))
        hip = ctx.enter_context(tc.tile_pool(name="hip", bufs=6))
        lop = ctx.enter_context(tc.tile_pool(name="lop", bufs=6))
        ohp = ctx.enter_context(tc.tile_pool(name="ohp", bufs=8))
        outp = ctx.enter_context(tc.tile_pool(name="outp", bufs=6))
        smp = ctx.enter_context(tc.tile_pool(name="smp", bufs=4))
        psb = ctx.enter_context(tc.tile_pool(name="psb", bufs=3, space="PSUM"))
        pst = ctx.enter_context(tc.tile_pool(name="pst", bufs=1, space="PSUM"))

        maxw = P * max(len(js) for q in range(SPC) for js in sched[q][0])
        iota_i = const.tile([P, maxw], i32)
        nc.gpsimd.iota(iota_i[:], pattern=[[1, maxw]], base=0, channel_multiplier=0)
        iota_w = const.tile([P, maxw], f32)
        nc.vector.tensor_copy(iota_w[:], iota_i[:])
        iota_f = iota_w[:, 0:P]
        pidx_i = const.tile([P, 1], i32)
        nc.gpsimd.iota(pidx_i[:], pattern=[[1, 1]], base=0, channel_multiplier=1)
        pidx_f = const.tile([P, 1], f32)
        nc.vector.tensor_copy(pidx_f[:], pidx_i[:])
        # identity[p, f] = (iota[f] == p), used by the PE transpose
        ident = const.tile([NST, NST], f32)
        nc.vector.tensor_scalar(ident[:], iota_f[0:NST, 0:NST], pidx_f[0:NST],
                                None, AO.is_equal)

        # segment ids for all 32 s-tiles of both slots -> [128, 32] per slot,
        # hoisted to the program start so the PE transpose clears early
        sid_alls = []
        for q in range(SPC):
            sid32_i = smp.tile([NST, P], i32, tag="sid32i", name=f"sid32i_{q}")
            nc.sync.dma_start(out=sid32_i[:],
                              in_=sid[q].rearrange("(n p) -> n p", p=P))
            sid32 = smp.tile([NST, P], f32, tag="sid32", name=f"sid32_{q}")
            nc.vector.tensor_copy(sid32[:], sid32_i[:])
            sidT_ps = pst.tile([P, NST], f32, tag="sidT", name=f"sidT_{q}")
            nc.tensor.transpose(sidT_ps[:], sid32[:], ident[:])
            sid_all = smp.tile([P, NST], f32, tag="sid_all", name=f"sid_all_{q}")
            nc.vector.tensor_copy(sid_all[:], sidT_ps[:])
            sid_alls.append(sid_all)

        for q in range(SPC):
            js_of, first_t, last_t = sched[q]
            first = dict(first_t)
            last = dict(last_t)
            sid_all = sid_alls[q]
            x_seq = x[q].rearrange("(n p) d -> p n d", p=P)  # [128, 32, 768]
            open_ps = {}
            for g in range(NST // SUPER):
                xt = xp.tile([P, SUPER, DW], f32)
                nc.sync.dma_start(out=xt[:, :, 0:D],
                                  in_=x_seq[:, g * SUPER:(g + 1) * SUPER, :])
                nc.vector.memset(xt[:, :, D:D + 1], 1.0)
                nc.vector.memset(xt[:, :, D + 1:DW], 0.0)
                hi = hip.tile([P, SUPER, DW], f32r)
                lo = lop.tile([P, SUPER, DW], f32r)
                # split per s-tile so downstream matmuls start earlier
                for h in range(SUPER):
                    nc.scalar.copy(hi[:, h, :], xt[:, h, :])  # -> fp32r
                    nc.vector.tensor_sub(lo[:, h, :], xt[:, h, :],
                                         hi[:, h, :])  # residual, rounded
                for si in range(SUPER):
                    i = g * SUPER + si
                    js = js_of[i]
                    # one is_equal over the whole 128*len(js) window; each
                    # 128-col slice is the one-hot for t-tile js[0]+k
                    ohw = ohp.tile([P, P * len(js)], f32r, tag="oh",
                                   name=f"oh_q{q}_i{i}")
                    nc.vector.tensor_scalar(
                        ohw[:], iota_w[:, 0:P * len(js)], float(js[0] * P),
                        sid_all[:, i:i + 1], AO.add, AO.is_equal)
                    for k, j in enumerate(js):
                        oh = ohw[:, k * P:(k + 1) * P]
                        st, sp_ = (first[j] == i), (last[j] == i)
                        if st:
                            open_ps[j] = (
                                psb.tile([P, DSPLIT], f32, tag="psA", bufs=4,
                                         name=f"accA_q{q}_j{j}"),
                                psb.tile([P, DW - DSPLIT], f32, tag="psB", bufs=3,
                                         name=f"accB_q{q}_j{j}"))
                        pa, pb = open_ps[j]
                        for term in (hi, lo):
                            t_first = st and term is hi
                            t_last = sp_ and term is lo
                            nc.tensor.matmul(pa[:], lhsT=oh[:],
                                             rhs=term[:, si, 0:DSPLIT],
                                             start=t_first, stop=t_last)
                            nc.tensor.matmul(pb[:], lhsT=oh[:],
                                             rhs=term[:, si, DSPLIT:DW],
                                             start=t_first, stop=t_last)
                        if sp_:
                            cnt = smp.tile([P, 1], f32, tag="cnt")
                            nc.vector.tensor_scalar_max(
                                cnt[:], pb[:, D - DSPLIT:D - DSPLIT + 1], 1.0)
                            rec = smp.tile([P, 1], f32, tag="rec")
                            nc.vector.reciprocal(rec[:], cnt[:])
                            ot = outp.tile([P, D], f32)
                            nc.scalar.activation(ot[:, 0:DSPLIT], pa[:],
                                                 mybir.ActivationFunctionType.Copy,
                                                 scale=rec[:])
                            nc.vector.tensor_scalar_mul(
                                ot[:, DSPLIT:D], pb[:, 0:D - DSPLIT], rec[:])
                            nc.sync.dma_start(out=out[q, j * P:(j + 1) * P, :],
                                              in_=ot[:])
                            del open_ps[j]
            # t-tiles no s-tile can touch: all-empty segments -> zeros
            for j in range(NTT):
                if j not in first:
                    zt = outp.tile([P, D], f32)
                    nc.vector.memset(zt[:], 0.0)
                    nc.sync.dma_start(out=out[q, j * P:(j + 1) * P, :], in_=zt[:])
    nc.compile()
    return nc


def _get_nc(segment_ids: np.ndarray):
    sched = _schedule(segment_ids)
    if sched not in _cache:
        _cache[sched] = _build(sched)
    return _cache[sched]


def run(raw_output, segment_ids, trace=False):
    from concourse.bass_utils import run_bass_kernel_spmd

    raw_output = np.ascontiguousarray(raw_output, dtype=np.float32)
    segment_ids = np.ascontiguousarray(segment_ids, dtype=np.int32)
    nc = _get_nc(segment_ids)
    in_maps = [{"raw_output": raw_output[c * SPC:(c + 1) * SPC],
                "segment_ids": segment_ids[c * SPC:(c + 1) * SPC]}
               for c in range(NCORES)]
    bkr = run_bass_kernel_spmd(nc, in_maps, list(range(NCORES)), trace=trace)
    full = np.concatenate([bkr.results[c]["out"] for c in range(NCORES)], axis=0)
    return full, bkr


def kernel(raw_output, segment_ids):
    full, _ = run(raw_output, segment_ids,
                  trace=bool(int(os.environ.get("KERNEL_TRACE", "0"))))
    return full


# revision 17
# speedup vs baseline: 1.0473x; 1.0473x over previous
"""Batched ragged segment-mean (BERTEmbedder merge loop) on 8 TRN2 NeuronCores.

Strategy
--------
Data-parallel over the batch: core c gets sequences [2c, 2c+1].  Within a
sequence, segment-sum is computed as a block-sparse one-hot matmul on the PE:

    out[t, d] = sum_s onehot[s, t] * x[s, d]

Segment ids are sorted per row, so each 128-subtoken tile only covers a
narrow window of token ids.  The host inspects the ids and builds a static
(s_tile, t_tile) pair schedule: for each 128-wide s-tile we emit matmuls only
into the 128-row t-tiles its ids can touch (union over the sequences that
share the SPMD program slot, so one program serves all 8 cores).  A column of
ones appended to the moving operand accumulates per-token counts in the same
PSUM tile; a reciprocal-multiply then turns sums into means.

fp32 matmul on TRN2 costs 4 PE cycles per output row (two half-speed passes).
Instead we run two 1-cycle fp32r matmuls: hi = round_fp32r(x) (11 mantissa
bits survive, measured) and lo = round_fp32r(x - hi), accumulated into the
same fp32 PSUM — reconstructing ~22 mantissa bits, indistinguishable from
fp32 at the output tolerance.  The one-hot (exactly representable) is built
on GPSIMD from a static iota and the per-partition segment id; segment ids
reach the partition dimension via one PE transpose per sequence.
"""

import os
import numpy as np

B, S, D, T, P = 16, 4096, 768, 2048, 128
NCORES = 8
SPC = B // NCORES          # sequences per core
NST, NTT = S // P, T // P  # 32 s-tiles, 16 t-tiles
DSPLIT = 512               # PSUM bank limit (fp32 words)
DW = D + 2                 # data + count-ones col + pad col (fp32r needs even N)
SUPER = 4                  # s-tiles per x-load DMA

_cache: dict = {}


def _schedule(segment_ids: np.ndarray):
    """Per program slot q: which t-tiles each s-tile touches, unioned over the
    sequences that run in that slot on every core (SPMD: one program)."""
    sched = []
    for q in range(SPC):
        seqs = [c * SPC + q for c in range(NCORES)]
        js_of = []
        for i in range(NST):
            blk = segment_ids[seqs, i * P:(i + 1) * P]
            lo, hi = int(blk.min()), int(blk.max())
            js_of.append(list(range(lo // P, hi // P + 1)))
        first, last = {}, {}
        for i in range(NST):
            for j in js_of[i]:
                first.setdefault(j, i)
                last[j] = i
        sched.append((tuple(tuple(js) for js in js_of),
                      tuple(sorted(first.items())),
                      tuple(sorted(last.items()))))
    return tuple(sched)


def _build(sched):
    from contextlib import ExitStack
    import concourse.bacc as bacc
    import concourse.tile as tile
    import concourse.mybir as mybir

    f32, f32r, i32 = mybir.dt.float32, mybir.dt.float32r, mybir.dt.int32
    AO = mybir.AluOpType
    nc = bacc.Bacc("TRN2", target_bir_lowering=False, debug=False)
    x = nc.dram_tensor("raw_output", [SPC, S, D], f32, kind="ExternalInput").ap()
    sid = nc.dram_tensor("segment_ids", [SPC, S], i32, kind="ExternalInput").ap()
    out = nc.dram_tensor("out", [SPC, T, D], f32, kind="ExternalOutput").ap()

    with ExitStack() as ctx:
        tc = ctx.enter_context(tile.TileContext(nc))
        const = ctx.enter_context(tc.tile_pool(name="const", bufs=1))
        xp = ctx.enter_context(tc.tile_pool(name="xp", bufs=4))
        hip = ctx.enter_context(tc.tile_pool(name="hip", bufs=4))
        lop = ctx.enter_context(tc.tile_pool(name="lop", bufs=4))
        ohp = ctx.enter_context(tc.tile_pool(name="ohp", bufs=8))
        outp = ctx.enter_context(tc.tile_pool(name="outp", bufs=6))
        smp = ctx.enter_context(tc.tile_pool(name="smp", bufs=4))
        psb = ctx.enter_context(tc.tile_pool(name="psb", bufs=3, space="PSUM"))
        pst = ctx.enter_context(tc.tile_pool(name="pst", bufs=1, space="PSUM"))

        maxw = P * max(len(js) for q in range(SPC) for js in sched[q][0])
        iota_i = const.tile([P, maxw], i32)
        nc.gpsimd.iota(iota_i[:], pattern=[[1, maxw]], base=0, channel_multiplier=0)
        iota_w = const.tile([P, maxw], f32)
        nc.vector.tensor_copy(iota_w[:], iota_i[:])
        iota_f = iota_w[:, 0:P]
        pidx_i = const.tile([P, 1], i32)
        nc.gpsimd.iota(pidx_i[:], pattern=[[1, 1]], base=0, channel_multiplier=1)
        pidx_f = const.tile([P, 1], f32)
        nc.vector.tensor_copy(pidx_f[:], pidx_i[:])
        # identity[p, f] = (iota[f] == p), used by the PE transpose
        ident = const.tile([NST, NST], f32)
        nc.vector.tensor_scalar(ident[:], iota_f[0:NST, 0:NST], pidx_f[0:NST],
                                None, AO.is_equal)

        # segment ids for all 32 s-tiles of both slots -> [128, 32] per slot,
        # hoisted to the program start so the PE transpose clears early
        sid_alls = []
        for q in range(SPC):
            sid32_i = smp.tile([NST, P], i32, tag="sid32i", name=f"sid32i_{q}")
            nc.sync.dma_start(out=sid32_i[:],
                              in_=sid[q].rearrange("(n p) -> n p", p=P))
            sid32 = smp.tile([NST, P], f32, tag="sid32", name=f"sid32_{q}")
            nc.vector.tensor_copy(sid32[:], sid32_i[:])
            sidT_ps = pst.tile([P, NST], f32, tag="sidT", name=f"sidT_{q}")
            nc.tensor.transpose(sidT_ps[:], sid32[:], ident[:])
            sid_all = smp.tile([P, NST], f32, tag="sid_all", name=f"sid_all_{q}")
            nc.vector.tensor_copy(sid_all[:], sidT_ps[:])
            sid_alls.append(sid_all)

        for q in range(SPC):
            js_of, first_t, last_t = sched[q]
            first = dict(first_t)
            last = dict(last_t)
            sid_all = sid_alls[q]
            x_seq = x[q].rearrange("(n p) d -> p n d", p=P)  # [128, 32, 768]
            open_ps = {}
            for g in range(NST // SUPER):
                xt = xp.tile([P, SUPER, DW], f32)
                nc.sync.dma_start(out=xt[:, :, 0:D],
                                  in_=x_seq[:, g * SUPER:(g + 1) * SUPER, :])
                nc.vector.memset(xt[:, :, D:D + 1], 1.0)
                nc.vector.memset(xt[:, :, D + 1:DW], 0.0)
                hi = hip.tile([P, SUPER, DW], f32r)
                lo = lop.tile([P, SUPER, DW], f32r)
                # split per half-super so downstream matmuls start earlier
                for h in range(0, SUPER, 2):
                    nc.scalar.copy(hi[:, h:h + 2, :], xt[:, h:h + 2, :])  # -> fp32r
                    nc.vector.tensor_sub(lo[:, h:h + 2, :], xt[:, h:h + 2, :],
                                         hi[:, h:h + 2, :])  # residual, rounded
                for si in range(SUPER):
                    i = g * SUPER + si
                    js = js_of[i]
                    # one is_equal over the whole 128*len(js) window; each
                    # 128-col slice is the one-hot for t-tile js[0]+k
                    ohw = ohp.tile([P, P * len(js)], f32r, tag="oh",
                                   name=f"oh_q{q}_i{i}")
                    nc.vector.tensor_scalar(
                        ohw[:], iota_w[:, 0:P * len(js)], float(js[0] * P),
                        sid_all[:, i:i + 1], AO.add, AO.is_equal)
                    for k, j in enumerate(js):
                        oh = ohw[:, k * P:(k + 1) * P]
                        st, sp_ = (first[j] == i), (last[j] == i)
                        if st:
                            open_ps[j] = (
                                psb.tile([P, DSPLIT], f32, tag="psA", bufs=4,
                                         name=f"accA_q{q}_j{j}"),
                                psb.tile([P, DW - DSPLIT], f32, tag="psB", bufs=3,
                                         name=f"accB_q{q}_j{j}"))
                        pa, pb = open_ps[j]
                        for term in (hi, lo):
                            t_first = st and term is hi
                            t_last = sp_ and term is lo
                            nc.tensor.matmul(pa[:], lhsT=oh[:],
                                             rhs=term[:, si, 0:DSPLIT],
                                             start=t_first, stop=t_last)
                            nc.tensor.matmul(pb[:], lhsT=oh[:],
                                             rhs=term[:, si, DSPLIT:DW],
                                             start=t_first, stop=t_last)
                        if sp_:
                            cnt = smp.tile([P, 1], f32, tag="cnt")
                            nc.vector.tensor_scalar_max(
                                cnt[:], pb[:, D - DSPLIT:D - DSPLIT + 1], 1.0)
                            rec = smp.tile([P, 1], f32, tag="rec")
                            nc.vector.reciprocal(rec[:], cnt[:])
                            ot = outp.tile([P, D], f32)
                            nc.scalar.activation(ot[:, 0:DSPLIT], pa[:],
                                                 mybir.ActivationFunctionType.Copy,
                                                 scale=rec[:])
                            nc.vector.tensor_scalar_mul(
                                ot[:, DSPLIT:D], pb[:, 0:D - DSPLIT], rec[:])
                            nc.sync.dma_start(out=out[q, j * P:(j + 1) * P, :],
                                              in_=ot[:])
                            del open_ps[j]
            # t-tiles no s-tile can touch: all-empty segments -> zeros
            for j in range(NTT):
                if j not in first:
                    zt = outp.tile([P, D], f32)
                    nc.vector.memset(zt[:], 0.0)
                    nc.sync.dma_start(out=out[q, j * P:(j + 1) * P, :], in_=zt[:])
    nc.compile()
    return nc


def _get_nc(segment_ids: np.ndarray):
    sched = _schedule(segment_ids)
    if sched not in _cache:
        _cache[sched] = _build(sched)
    return _cache[sched]


def run(raw_output, segment_ids, trace=False):
    from concourse.bass_utils import run_bass_kernel_spmd

    raw_output = np.ascontiguousarray(raw_output, dtype=np.float32)
    segment_ids = np.ascontiguousarray(segment_ids, dtype=np.int32)
    nc = _get_nc(segment_ids)
    in_maps = [{"raw_output": raw_output[c * SPC:(c + 1) * SPC],
                "segment_ids": segment_ids[c * SPC:(c + 1) * SPC]}
               for c in range(NCORES)]
    bkr = run_bass_kernel_spmd(nc, in_maps, list(range(NCORES)), trace=trace)
    full = np.concatenate([bkr.results[c]["out"] for c in range(NCORES)], axis=0)
    return full, bkr


def kernel(raw_output, segment_ids):
    full, _ = run(raw_output, segment_ids,
                  trace=bool(int(os.environ.get("KERNEL_TRACE", "0"))))
    return full
